# revision 19
# baseline (speedup 1.0000x reference)
"""Trainium2 Bass kernel for nn_MinibatchDiscriminator.

reference:
    M = (x @ T).reshape(B, OUT_F, KD)
    norm[i, j, o] = sum_k |M[i,o,k] - M[j,o,k]|
    oX[j, o] = sum_i exp(-norm[i,j,o])
    out = concat(x, oX, axis=1)

Sharding: batch dim of the j-loop across 8 cores. Each core receives a
batch-rotated copy of x^T (so its own 128 j-rows are always M_T columns
0..127 -- one SPMD program serves all cores), computes the full
M_T = (x_rot @ T)^T in [ok, i] layout on the PE.

Symmetry: exp(-norm) is symmetric in (i, j), so each core only computes
i in [0, 640) local (its own diagonal block, neighbours d=1..3, and the
d=4 block which both endpoint cores compute for their own rows). For
d=1..3 the per-(o, i) column sums over the core's j rows are also
accumulated (tile SACC) and redistributed to the i-owning shards during
host-side assembly; the diagonal block contains both (i,j) orders and
the d=4 block is computed by both endpoints, so neither contributes
column sums. Per j:
  |M_T - M_T[:, j]|  one-pass on DVE (tensor_scalar sub+abs_max) and
                     ACT (Abs activation with per-partition bias), split
                     across both engines for throughput
  k-group reduce     PE matmul with a block-ones selector (partition
                     groups of KD=8 -> OUT_F rows), j-pairs packed two
                     per PSUM tile
  exp + i-sum        single ACT Exp(scale=-1) with accum_out
"""

import ml_dtypes
import numpy as np

import concourse.bacc as bacc
import concourse.bass as bass
import concourse.mybir as mybir
import concourse.tile as tile

B, IN_F, OUT_F, KD = 1024, 1024, 50, 8
OK = OUT_F * KD  # 400
NCORE = 8
JS = B // NCORE  # 128 rows of the batch per core
P = 128
F32 = mybir.dt.float32
BF16 = mybir.dt.bfloat16

# ok-partition chunks: [lo, hi) over the 400 (o,k) pairs, o-major
CHUNKS = [(0, 128), (128, 256), (256, 384), (384, 400)]
IW = 640  # i-range computed per core (5 of 8 blocks, symmetry)
# matmul free-dim slices of the i-range (<=512 each, psum-bank aligned)
HS = [(0, 512), (512, 640)]
# (jsub, chunk) pairs generated on ACT; the rest go to DVE. DVE does
# subtract (bf16, 4x-eligible) + packed sign-mask AND on a uint32 view;
# ACT does Abs(x + bias) in one pass and also owns the exp stage.
ACT_GEN = {(0, 1), (1, 2), (0, 3)}
ACT_GEN_ODD = {(0, 1), (1, 2), (1, 0)}


def _build_nc():
    nc = bacc.Bacc(
        "TRN2",
        target_bir_lowering=False,
        debug=False,
        num_devices=NCORE,
    )
    xT = nc.dram_tensor("xT", [IN_F, B], BF16, kind="ExternalInput").ap()
    xj = nc.dram_tensor("xj", [JS, IN_F], F32, kind="ExternalInput").ap()
    t_in = nc.dram_tensor("T", [IN_F, OK], BF16, kind="ExternalInput").ap()
    sel_in = nc.dram_tensor("sel", [P, 256], BF16, kind="ExternalInput").ap()
    xout = nc.dram_tensor("xout", [JS, IN_F], F32, kind="ExternalOutput").ap()
    ox_out = nc.dram_tensor("oxpair", [P, 64], F32, kind="ExternalOutput").ap()
    s_out = nc.dram_tensor("sacc", [P, 384], F32, kind="ExternalOutput").ap()

    sub = mybir.AluOpType.subtract

    with tile.TileContext(nc) as tc:
        with (
            tc.tile_pool(name="const", bufs=1) as cpool,
            tc.tile_pool(name="xtp", bufs=1) as xtpool,
            tc.tile_pool(name="agen", bufs=16) as apool,
        ):
            # x rows pass through to the output unchanged
            nc.sync.dma_start(out=xout, in_=xj)

            sel_sb = cpool.tile([P, 256], BF16)
            nc.sync.dma_start(out=sel_sb[:], in_=sel_in)

            t_sb = []
            xt_sb = []
            for fc in range(8):
                tt = cpool.tile([P, OK], BF16, tag=f"t{fc}")
                nc.sync.dma_start(out=tt[:], in_=t_in[fc * 128 : (fc + 1) * 128, :])
                t_sb.append(tt)
                xtt = xtpool.tile([P, B], BF16, tag=f"xt{fc}")
                nc.sync.dma_start(out=xtt[:], in_=xT[fc * 128 : (fc + 1) * 128, :])
                xt_sb.append(xtt)

            # M_T chunks [128, 1024] in bf16 (+ negated copy for the ACT
            # bias path). bf16 is safe: the smallest cross-pair L1 norm is
            # ~50 while exp(-norm) only registers against the exact self
            # term below norm ~16, so +-2 of bf16 noise cannot surface.
            mtb = [cpool.tile([P, B], BF16, tag=f"mtb{c}", name=f"mtb{c}") for c in range(4)]
            # negated fp32 copy OF THE BF16 VALUES (exact upcast) for the
            # per-partition scalar/bias operands, which must be fp32; using
            # raw-fp32 M here would break the exact-zero self term. The DVE
            # path uses op0=add with this negated column.
            nmt32 = [cpool.tile([P, B], F32, tag=f"nmt32{c}", name=f"nmt32{c}") for c in range(4)]
            # chunk 3 only has 16 live rows; zero the rest so the garbage
            # partitions reduce to norm=0 -> exp(0)=1 in ignored psum rows
            nc.vector.memset(mtb[3][:], 0.0)
            nc.vector.memset(nmt32[3][:], 0.0)

            with tc.tile_pool(name="psmt", bufs=2, space=bass.MemorySpace.PSUM) as psmt:
                for c, (lo, hi) in enumerate(CHUNKS):
                    w = hi - lo
                    for h in range(2):
                        ps = psmt.tile([P, 512], F32, tag="psmt")
                        for fc in range(8):
                            nc.tensor.matmul(
                                ps[0:w, :],
                                t_sb[fc][:, lo:hi],
                                xt_sb[fc][:, h * 512 : (h + 1) * 512],
                                start=(fc == 0),
                                stop=(fc == 7),
                            )
                        sl = slice(h * 512, (h + 1) * 512)
                        nc.vector.tensor_copy(mtb[c][0:w, sl], ps[0:w, :])
                        nc.scalar.mul(nmt32[c][0:w, sl], mtb[c][0:w, sl], -1.0)

            oxacc = cpool.tile([P, 64], F32)
            sacc = cpool.tile([P, 384], F32)
            nc.vector.memset(sacc[:], 0.0)

            main_pools = tc.tile_pool(name="psn", bufs=4, space=bass.MemorySpace.PSUM)
            psn = main_pools.__enter__()
            epool_cm = tc.tile_pool(name="esc", bufs=4)
            epool = epool_cm.__enter__()

            # main loop: j-pairs (p, p+64) share one [128, IW] psum tile
            for pr in range(64):
                ps = psn.tile([P, IW], F32, tag="psn")
                act_set = ACT_GEN_ODD if (pr % 2) else ACT_GEN
                for jsub in range(2):
                    j = pr + 64 * jsub
                    r0 = 64 * jsub
                    for c in range(4):
                        a = apool.tile([P, IW], BF16, tag="A")
                        if (jsub, c) in act_set:
                            nc.scalar.activation(
                                a[:],
                                mtb[c][:, 0:IW],
                                mybir.ActivationFunctionType.Abs,
                                bias=nmt32[c][:, j : j + 1],
                                scale=1.0,
                            )
                        else:
                            nc.vector.tensor_scalar(
                                a[:],
                                mtb[c][:, 0:IW],
                                nmt32[c][:, j : j + 1],
                                None,
                                op0=mybir.AluOpType.add,
                            )
                            au = a.bitcast(mybir.dt.uint16)
                            nc.vector.tensor_scalar(
                                au[:],
                                au[:],
                                0x7FFF,
                                None,
                                op0=mybir.AluOpType.bitwise_and,
                            )
                        # chunk c's selector scatters its 16 o-groups to rows
                        # 16c..16c+15 of the 64-row block; 4 chunks accumulate
                        for lo, hi in HS:
                            nc.tensor.matmul(
                                ps[r0 : r0 + 64, lo:hi],
                                sel_sb[:, 64 * c : 64 * (c + 1)],
                                a[:, lo:hi],
                                start=(c == 0),
                                stop=(c == 3),
                            )
                e = epool.tile([P, IW], BF16, tag="E")
                nc.scalar.activation(
                    e[:],
                    ps[:],
                    mybir.ActivationFunctionType.Exp,
                    bias=0.0,
                    scale=-1.0,
                    accum_out=oxacc[:, pr : pr + 1],
                )
                # transpose contributions for the d=1..3 i-blocks
                nc.vector.tensor_tensor(
                    sacc[:],
                    sacc[:],
                    e[:, 128:512],
                    op=mybir.AluOpType.add,
                )

            nc.sync.dma_start(out=ox_out, in_=oxacc[:])
            nc.sync.dma_start(out=s_out, in_=sacc[:])
            epool_cm.__exit__(None, None, None)
            main_pools.__exit__(None, None, None)

    nc.compile()
    return nc


_NC = None


def _get_nc():
    global _NC
    if _NC is None:
        _NC = _build_nc()
    return _NC


def _make_in_maps(x, t):
    x = np.ascontiguousarray(np.asarray(x, dtype=np.float32))
    t16 = np.ascontiguousarray(np.asarray(t, dtype=np.float32).astype(ml_dtypes.bfloat16))
    xtg = np.ascontiguousarray(x.T.astype(ml_dtypes.bfloat16))
    # per-chunk selectors: chunk c maps partition p (= ok - 128c) to o-row
    # 16c + p // KD of the 64-row psum block
    sel = np.zeros((P, 256), dtype=ml_dtypes.bfloat16)
    for c in range(4):
        for g in range(16):
            sel[g * KD : (g + 1) * KD, 64 * c + 16 * c + g] = 1.0
    in_maps = []
    for c in range(NCORE):
        in_maps.append(
            {
                "xT": np.ascontiguousarray(np.roll(xtg, -c * JS, axis=1)),
                "xj": np.ascontiguousarray(x[c * JS : (c + 1) * JS]),
                "T": t16,
                "sel": sel,
            }
        )
    return in_maps


def _assemble(results):
    out = np.empty((B, IN_F + OUT_F), dtype=np.float32)
    oX = np.zeros((B, OUT_F), dtype=np.float32)
    for c in range(NCORE):
        r = results[c]
        rows = slice(c * JS, (c + 1) * JS)
        out[rows, :IN_F] = r["xout"]
        oxp = r["oxpair"]  # [128, 64]: rows 0:50 -> j=pr, rows 64:114 -> j=pr+64
        oX[rows] += np.concatenate(
            [oxp[0:OUT_F, :].T, oxp[64 : 64 + OUT_F, :].T], axis=0
        )
        # transpose contributions: sacc[(jsub, o), t] sums exp terms over this
        # core's j rows for local i = 128 + t (the d=1..3 blocks)
        s = r["sacc"]
        s50 = (s[0:OUT_F, :] + s[64 : 64 + OUT_F, :]).T  # [384, 50]
        g0 = (c + 1) * JS
        for blk in range(3):
            gs = (g0 + blk * JS) % B
            oX[gs : gs + JS] += s50[blk * JS : (blk + 1) * JS]
    out[:, IN_F:] = oX
    return out


def kernel(x, T):
    from concourse.bass_utils import run_bass_kernel_spmd

    nc = _get_nc()
    in_maps = _make_in_maps(x, T)
    res = run_bass_kernel_spmd(nc, in_maps, core_ids=list(range(NCORE)))
    return _assemble(res.results)


def _ensure_ntff_hook():
    """The agent image's antenv lacks axon_hooks; synthesize it from the
    ctypes NTFF driver in trn_agent_boot so trace=True works."""
    import sys
    import types

    try:
        from antenv.axon_hooks import get_axon_ntff_profile_hook  # noqa: F401

        return
    except ImportError:
        pass
    from trn_agent_boot.trn_boot import _ntff_profile_via_ctypes

    hook = _ntff_profile_via_ctypes("/opt/axon/libaxon_pjrt.so")
    mod = types.ModuleType("antenv.axon_hooks")
    mod.get_axon_ntff_profile_hook = lambda: hook
    mod.set_axon_ntff_profile_hook = lambda h: None
    sys.modules["antenv.axon_hooks"] = mod


def kernel_profiled(x, T, tmpdir=None):
    """Same as kernel() but with NTFF tracing; returns (out, exec_time_ns)."""
    import concourse.bass_utils as bu

    _ensure_ntff_hook()
    bu.upload_artifacts = lambda d: d  # no S3 in this container

    nc = _get_nc()
    in_maps = _make_in_maps(x, T)
    res = bu.run_bass_kernel_spmd(
        nc, in_maps, core_ids=list(range(NCORE)), trace=True, tmpdir=tmpdir
    )
    return _assemble(res.results), res.exec_time_ns


# revision 21
# speedup vs baseline: 1.0183x; 1.0183x over previous
"""Trainium2 Bass kernel for nn_MinibatchDiscriminator.

reference:
    M = (x @ T).reshape(B, OUT_F, KD)
    norm[i, j, o] = sum_k |M[i,o,k] - M[j,o,k]|
    oX[j, o] = sum_i exp(-norm[i,j,o])
    out = concat(x, oX, axis=1)

Sharding: batch dim of the j-loop across 8 cores. Each core receives a
batch-rotated copy of x^T (so its own 128 j-rows are always M_T columns
0..127 -- one SPMD program serves all cores), computes the full
M_T = (x_rot @ T)^T in [ok, i] layout on the PE.

Symmetry: exp(-norm) is symmetric in (i, j), so each core only computes
i in [0, 640) local (its own diagonal block, neighbours d=1..3, and the
d=4 block which both endpoint cores compute for their own rows). For
d=1..3 the per-(o, i) column sums over the core's j rows are also
accumulated (tile SACC) and redistributed to the i-owning shards during
host-side assembly; the diagonal block contains both (i,j) orders and
the d=4 block is computed by both endpoints, so neither contributes
column sums. Per j:
  |M_T - M_T[:, j]|  split across DVE (tensor_scalar add of the negated
                     column, then a sign-bit bitwise_and on a packed
                     uint32 view -- the ISA has no float abs ALU op) and
                     ACT (Abs activation with per-partition bias)
  k-group reduce     PE matmul with a block-ones selector (partition
                     groups of KD=8 -> OUT_F rows), j-pairs packed two
                     per PSUM tile
  exp + i-sum        single ACT Exp(scale=-1) with accum_out
"""

import ml_dtypes
import numpy as np

import concourse.bacc as bacc
import concourse.bass as bass
import concourse.mybir as mybir
import concourse.tile as tile

B, IN_F, OUT_F, KD = 1024, 1024, 50, 8
OK = OUT_F * KD  # 400
NCORE = 8
JS = B // NCORE  # 128 rows of the batch per core
P = 128
F32 = mybir.dt.float32
BF16 = mybir.dt.bfloat16

# ok-partition chunks: [lo, hi) over the 400 (o,k) pairs, o-major
CHUNKS = [(0, 128), (128, 256), (256, 384), (384, 400)]
IW = 640  # i-range computed per core (5 of 8 blocks, symmetry)
# matmul free-dim slices of the i-range (<=512 each, psum-bank aligned)
HS = [(0, 512), (512, 640)]
# (jsub, chunk) pairs generated on ACT; the rest go to DVE. DVE does
# subtract (bf16, 4x-eligible) + packed sign-mask AND on a uint32 view;
# ACT does Abs(x + bias) in one pass and also owns the exp stage.
ACT_GEN = {(0, 1), (1, 2), (0, 3)}
ACT_GEN_ODD = {(0, 1), (1, 2), (1, 0)}


def _build_nc():
    nc = bacc.Bacc(
        "TRN2",
        target_bir_lowering=False,
        debug=False,
        num_devices=NCORE,
    )
    xT = nc.dram_tensor("xT", [IN_F, B], BF16, kind="ExternalInput").ap()
    xj = nc.dram_tensor("xj", [JS, IN_F], F32, kind="ExternalInput").ap()
    t_in = nc.dram_tensor("T", [IN_F, OK], BF16, kind="ExternalInput").ap()
    sel_in = nc.dram_tensor("sel", [P, 256], BF16, kind="ExternalInput").ap()
    xout = nc.dram_tensor("xout", [JS, IN_F], F32, kind="ExternalOutput").ap()
    ox_out = nc.dram_tensor("oxpair", [P, 64], F32, kind="ExternalOutput").ap()
    s_out = nc.dram_tensor("sacc", [P, 384], F32, kind="ExternalOutput").ap()

    sub = mybir.AluOpType.subtract

    with tile.TileContext(nc) as tc:
        with (
            tc.tile_pool(name="const", bufs=1) as cpool,
            tc.tile_pool(name="xtp", bufs=1) as xtpool,
            tc.tile_pool(name="agen", bufs=16) as apool,
            tc.tile_pool(name="psn", bufs=3, space=bass.MemorySpace.PSUM) as psn,
            tc.tile_pool(name="esc", bufs=3) as epool,
        ):
            # x rows pass through to the output unchanged
            nc.sync.dma_start(out=xout, in_=xj)

            sel_sb = cpool.tile([P, 256], BF16)
            nc.sync.dma_start(out=sel_sb[:], in_=sel_in)

            t_sb = []
            xt_sb = []
            for fc in range(8):
                tt = cpool.tile([P, OK], BF16, tag=f"t{fc}")
                nc.sync.dma_start(out=tt[:], in_=t_in[fc * 128 : (fc + 1) * 128, :])
                t_sb.append(tt)
                xtt = xtpool.tile([P, B], BF16, tag=f"xt{fc}")
                nc.sync.dma_start(out=xtt[:], in_=xT[fc * 128 : (fc + 1) * 128, :])
                xt_sb.append(xtt)

            # M_T chunks [128, 1024] in bf16 (+ negated copy for the ACT
            # bias path). bf16 is safe: the smallest cross-pair L1 norm is
            # ~50 while exp(-norm) only registers against the exact self
            # term below norm ~16, so +-2 of bf16 noise cannot surface.
            mtb = [cpool.tile([P, B], BF16, tag=f"mtb{c}", name=f"mtb{c}") for c in range(4)]
            # negated fp32 copy OF THE BF16 VALUES (exact upcast) for the
            # per-partition scalar/bias operands, which must be fp32; using
            # raw-fp32 M here would break the exact-zero self term. The DVE
            # path uses op0=add with this negated column.
            nmt32 = [cpool.tile([P, B], F32, tag=f"nmt32{c}", name=f"nmt32{c}") for c in range(4)]
            # chunk 3 only has 16 live rows; zero the rest so the garbage
            # partitions reduce to norm=0 -> exp(0)=1 in ignored psum rows
            nc.vector.memset(mtb[3][:], 0.0)
            nc.vector.memset(nmt32[3][:], 0.0)

            with tc.tile_pool(name="psmt", bufs=2, space=bass.MemorySpace.PSUM) as psmt:
                for c, (lo, hi) in enumerate(CHUNKS):
                    w = hi - lo
                    for h in range(2):
                        ps = psmt.tile([P, 512], F32, tag="psmt")
                        for fc in range(8):
                            nc.tensor.matmul(
                                ps[0:w, :],
                                t_sb[fc][:, lo:hi],
                                xt_sb[fc][:, h * 512 : (h + 1) * 512],
                                start=(fc == 0),
                                stop=(fc == 7),
                            )
                        sl = slice(h * 512, (h + 1) * 512)
                        nc.vector.tensor_copy(mtb[c][0:w, sl], ps[0:w, :])
                        nc.scalar.mul(nmt32[c][0:w, sl], mtb[c][0:w, sl], -1.0)

            oxacc = cpool.tile([P, 64], F32)
            sacc = cpool.tile([P, 384], F32)
            sacc_b = cpool.tile([P, 384], F32)
            nc.vector.memset(sacc[:], 0.0)
            nc.vector.memset(sacc_b[:], 0.0)

            # main loop: j-pairs (p, p+64) share one [128, IW] psum tile
            for pr in range(64):
                ps = psn.tile([P, IW], F32, tag="psn")
                act_set = ACT_GEN_ODD if (pr % 2) else ACT_GEN
                for jsub in range(2):
                    j = pr + 64 * jsub
                    r0 = 64 * jsub
                    for c in range(4):
                        a = apool.tile([P, IW], BF16, tag="A")
                        if (jsub, c) in act_set:
                            nc.scalar.activation(
                                a[:],
                                mtb[c][:, 0:IW],
                                mybir.ActivationFunctionType.Abs,
                                bias=nmt32[c][:, j : j + 1],
                                scale=1.0,
                            )
                        else:
                            nc.vector.tensor_scalar(
                                a[:],
                                mtb[c][:, 0:IW],
                                nmt32[c][:, j : j + 1],
                                None,
                                op0=mybir.AluOpType.add,
                            )
                            au = a.bitcast(mybir.dt.uint16)
                            nc.vector.tensor_scalar(
                                au[:],
                                au[:],
                                0x7FFF,
                                None,
                                op0=mybir.AluOpType.bitwise_and,
                            )
                        # chunk c's selector scatters its 16 o-groups to rows
                        # 16c..16c+15 of the 64-row block; 4 chunks accumulate
                        for lo, hi in HS:
                            nc.tensor.matmul(
                                ps[r0 : r0 + 64, lo:hi],
                                sel_sb[:, 64 * c : 64 * (c + 1)],
                                a[:, lo:hi],
                                start=(c == 0),
                                stop=(c == 3),
                            )
                e = epool.tile([P, IW], BF16, tag="E")
                nc.scalar.activation(
                    e[:],
                    ps[:],
                    mybir.ActivationFunctionType.Exp,
                    bias=0.0,
                    scale=-1.0,
                    accum_out=oxacc[:, pr : pr + 1],
                )
                # transpose contributions for the d=1..3 i-blocks; two
                # accumulators so consecutive pairs' DVE adds don't chain
                st = sacc if (pr % 2 == 0) else sacc_b
                nc.vector.tensor_tensor(
                    st[:],
                    st[:],
                    e[:, 128:512],
                    op=mybir.AluOpType.add,
                )

            nc.vector.tensor_tensor(
                sacc[:], sacc[:], sacc_b[:], op=mybir.AluOpType.add
            )
            nc.sync.dma_start(out=ox_out, in_=oxacc[:])
            nc.sync.dma_start(out=s_out, in_=sacc[:])

    nc.compile()
    return nc


_NC = None


def _get_nc():
    global _NC
    if _NC is None:
        _NC = _build_nc()
    return _NC


def _make_in_maps(x, t):
    x = np.ascontiguousarray(np.asarray(x, dtype=np.float32))
    t16 = np.ascontiguousarray(np.asarray(t, dtype=np.float32).astype(ml_dtypes.bfloat16))
    xtg = np.ascontiguousarray(x.T.astype(ml_dtypes.bfloat16))
    # per-chunk selectors: chunk c maps partition p (= ok - 128c) to o-row
    # 16c + p // KD of the 64-row psum block
    sel = np.zeros((P, 256), dtype=ml_dtypes.bfloat16)
    for c in range(4):
        for g in range(16):
            sel[g * KD : (g + 1) * KD, 64 * c + 16 * c + g] = 1.0
    in_maps = []
    for c in range(NCORE):
        in_maps.append(
            {
                "xT": np.ascontiguousarray(np.roll(xtg, -c * JS, axis=1)),
                "xj": np.ascontiguousarray(x[c * JS : (c + 1) * JS]),
                "T": t16,
                "sel": sel,
            }
        )
    return in_maps


def _assemble(results):
    out = np.empty((B, IN_F + OUT_F), dtype=np.float32)
    oX = np.zeros((B, OUT_F), dtype=np.float32)
    for c in range(NCORE):
        r = results[c]
        rows = slice(c * JS, (c + 1) * JS)
        out[rows, :IN_F] = r["xout"]
        oxp = r["oxpair"]  # [128, 64]: rows 0:50 -> j=pr, rows 64:114 -> j=pr+64
        oX[rows] += np.concatenate(
            [oxp[0:OUT_F, :].T, oxp[64 : 64 + OUT_F, :].T], axis=0
        )
        # transpose contributions: sacc[(jsub, o), t] sums exp terms over this
        # core's j rows for local i = 128 + t (the d=1..3 blocks)
        s = r["sacc"]
        s50 = (s[0:OUT_F, :] + s[64 : 64 + OUT_F, :]).T  # [384, 50]
        g0 = (c + 1) * JS
        for blk in range(3):
            gs = (g0 + blk * JS) % B
            oX[gs : gs + JS] += s50[blk * JS : (blk + 1) * JS]
    out[:, IN_F:] = oX
    return out


def kernel(x, T):
    from concourse.bass_utils import run_bass_kernel_spmd

    nc = _get_nc()
    in_maps = _make_in_maps(x, T)
    res = run_bass_kernel_spmd(nc, in_maps, core_ids=list(range(NCORE)))
    return _assemble(res.results)


def _ensure_ntff_hook():
    """The agent image's antenv lacks axon_hooks; synthesize it from the
    ctypes NTFF driver in trn_agent_boot so trace=True works."""
    import sys
    import types

    try:
        from antenv.axon_hooks import get_axon_ntff_profile_hook  # noqa: F401

        return
    except ImportError:
        pass
    from trn_agent_boot.trn_boot import _ntff_profile_via_ctypes

    hook = _ntff_profile_via_ctypes("/opt/axon/libaxon_pjrt.so")
    mod = types.ModuleType("antenv.axon_hooks")
    mod.get_axon_ntff_profile_hook = lambda: hook
    mod.set_axon_ntff_profile_hook = lambda h: None
    sys.modules["antenv.axon_hooks"] = mod


def kernel_profiled(x, T, tmpdir=None):
    """Same as kernel() but with NTFF tracing; returns (out, exec_time_ns)."""
    import concourse.bass_utils as bu

    _ensure_ntff_hook()
    bu.upload_artifacts = lambda d: d  # no S3 in this container

    nc = _get_nc()
    in_maps = _make_in_maps(x, T)
    res = bu.run_bass_kernel_spmd(
        nc, in_maps, core_ids=list(range(NCORE)), trace=True, tmpdir=tmpdir
    )
    return _assemble(res.results), res.exec_time_ns


# revision 22
# speedup vs baseline: 1.0378x; 1.0191x over previous
"""Trainium2 Bass kernel for nn_MinibatchDiscriminator.

reference:
    M = (x @ T).reshape(B, OUT_F, KD)
    norm[i, j, o] = sum_k |M[i,o,k] - M[j,o,k]|
    oX[j, o] = sum_i exp(-norm[i,j,o])
    out = concat(x, oX, axis=1)

Sharding: batch dim of the j-loop across 8 cores. Each core receives a
batch-rotated copy of x^T (so its own 128 j-rows are always M_T columns
0..127 -- one SPMD program serves all cores), computes the full
M_T = (x_rot @ T)^T in [ok, i] layout on the PE.

Symmetry: exp(-norm) is symmetric in (i, j), so each core only computes
i in [0, 640) local (its own diagonal block, neighbours d=1..3, and the
d=4 block which both endpoint cores compute for their own rows). For
d=1..3 the per-(o, i) column sums over the core's j rows are also
accumulated (tile SACC) and redistributed to the i-owning shards during
host-side assembly; the diagonal block contains both (i,j) orders and
the d=4 block is computed by both endpoints, so neither contributes
column sums. Per j:
  |M_T - M_T[:, j]|  split across DVE (tensor_scalar add of the negated
                     column, then a sign-bit bitwise_and on a packed
                     uint32 view -- the ISA has no float abs ALU op) and
                     ACT (Abs activation with per-partition bias)
  k-group reduce     PE matmul with a block-ones selector (partition
                     groups of KD=8 -> OUT_F rows), j-pairs packed two
                     per PSUM tile
  exp + i-sum        single ACT Exp(scale=-1) with accum_out
"""

import ml_dtypes
import numpy as np

import concourse.bacc as bacc
import concourse.bass as bass
import concourse.mybir as mybir
import concourse.tile as tile

B, IN_F, OUT_F, KD = 1024, 1024, 50, 8
OK = OUT_F * KD  # 400
NCORE = 8
JS = B // NCORE  # 128 rows of the batch per core
P = 128
F32 = mybir.dt.float32
BF16 = mybir.dt.bfloat16

# ok-partition chunks: [lo, hi) over the 400 (o,k) pairs, o-major
CHUNKS = [(0, 128), (128, 256), (256, 384), (384, 400)]
IW = 640  # i-range computed per core (5 of 8 blocks, symmetry)
# matmul free-dim slices of the i-range (<=512 each, psum-bank aligned)
HS = [(0, 512), (512, 640)]
# (jsub, chunk) pairs generated on ACT; the rest go to DVE. DVE does
# subtract (bf16, 4x-eligible) + packed sign-mask AND on a uint32 view;
# ACT does Abs(x + bias) in one pass and also owns the exp stage.
ACT_GEN = {(0, 1), (1, 2), (0, 3)}
ACT_GEN_ODD = {(0, 1), (1, 2), (1, 0)}


def _build_nc():
    nc = bacc.Bacc(
        "TRN2",
        target_bir_lowering=False,
        debug=False,
        num_devices=NCORE,
    )
    xT = nc.dram_tensor("xT", [IN_F, B], BF16, kind="ExternalInput").ap()
    xj = nc.dram_tensor("xj", [JS, IN_F], F32, kind="ExternalInput").ap()
    t_in = nc.dram_tensor("T", [IN_F, OK], BF16, kind="ExternalInput").ap()
    sel_in = nc.dram_tensor("sel", [P, 320], BF16, kind="ExternalInput").ap()
    xout = nc.dram_tensor("xout", [JS, IN_F], F32, kind="ExternalOutput").ap()
    ox_out = nc.dram_tensor("oxpair", [P, 64], F32, kind="ExternalOutput").ap()
    s_out = nc.dram_tensor("sacc", [64, 384], F32, kind="ExternalOutput").ap()

    sub = mybir.AluOpType.subtract

    with tile.TileContext(nc) as tc:
        with (
            tc.tile_pool(name="const", bufs=1) as cpool,
            tc.tile_pool(name="xtp", bufs=1) as xtpool,
            tc.tile_pool(name="agen", bufs=16) as apool,
            tc.tile_pool(name="psn", bufs=3, space=bass.MemorySpace.PSUM) as psn,
            tc.tile_pool(name="spool", bufs=1, space=bass.MemorySpace.PSUM) as spool,
            tc.tile_pool(name="esc", bufs=3) as epool,
        ):
            # x rows pass through to the output unchanged
            nc.sync.dma_start(out=xout, in_=xj)

            sel_sb = cpool.tile([P, 320], BF16)
            nc.sync.dma_start(out=sel_sb[:], in_=sel_in)

            t_sb = []
            xt_sb = []
            for fc in range(8):
                tt = cpool.tile([P, OK], BF16, tag=f"t{fc}")
                nc.sync.dma_start(out=tt[:], in_=t_in[fc * 128 : (fc + 1) * 128, :])
                t_sb.append(tt)
                xtt = xtpool.tile([P, B], BF16, tag=f"xt{fc}")
                nc.sync.dma_start(out=xtt[:], in_=xT[fc * 128 : (fc + 1) * 128, :])
                xt_sb.append(xtt)

            # M_T chunks [128, 1024] in bf16 (+ negated copy for the ACT
            # bias path). bf16 is safe: the smallest cross-pair L1 norm is
            # ~50 while exp(-norm) only registers against the exact self
            # term below norm ~16, so +-2 of bf16 noise cannot surface.
            mtb = [cpool.tile([P, B], BF16, tag=f"mtb{c}", name=f"mtb{c}") for c in range(4)]
            # negated fp32 copy OF THE BF16 VALUES (exact upcast) for the
            # per-partition scalar/bias operands, which must be fp32; using
            # raw-fp32 M here would break the exact-zero self term. The DVE
            # path uses op0=add with this negated column.
            nmt32 = [cpool.tile([P, B], F32, tag=f"nmt32{c}", name=f"nmt32{c}") for c in range(4)]
            # chunk 3 only has 16 live rows; zero the rest so the garbage
            # partitions reduce to norm=0 -> exp(0)=1 in ignored psum rows
            nc.vector.memset(mtb[3][:], 0.0)
            nc.vector.memset(nmt32[3][:], 0.0)

            with tc.tile_pool(name="psmt", bufs=1, space=bass.MemorySpace.PSUM) as psmt:
                for c, (lo, hi) in enumerate(CHUNKS):
                    w = hi - lo
                    for h in range(2):
                        ps = psmt.tile([P, 512], F32, tag="psmt")
                        for fc in range(8):
                            nc.tensor.matmul(
                                ps[0:w, :],
                                t_sb[fc][:, lo:hi],
                                xt_sb[fc][:, h * 512 : (h + 1) * 512],
                                start=(fc == 0),
                                stop=(fc == 7),
                            )
                        sl = slice(h * 512, (h + 1) * 512)
                        nc.vector.tensor_copy(mtb[c][0:w, sl], ps[0:w, :])
                        nc.scalar.mul(nmt32[c][0:w, sl], mtb[c][0:w, sl], -1.0)

            oxacc = cpool.tile([P, 64], F32)
            psum_s = spool.tile([64, 384], F32)

            # main loop: j-pairs (p, p+64) share one [128, IW] psum tile
            for pr in range(64):
                ps = psn.tile([P, IW], F32, tag="psn")
                act_set = ACT_GEN_ODD if (pr % 2) else ACT_GEN
                for jsub in range(2):
                    j = pr + 64 * jsub
                    r0 = 64 * jsub
                    for c in range(4):
                        a = apool.tile([P, IW], BF16, tag="A")
                        if (jsub, c) in act_set:
                            nc.scalar.activation(
                                a[:],
                                mtb[c][:, 0:IW],
                                mybir.ActivationFunctionType.Abs,
                                bias=nmt32[c][:, j : j + 1],
                                scale=1.0,
                            )
                        else:
                            nc.vector.tensor_scalar(
                                a[:],
                                mtb[c][:, 0:IW],
                                nmt32[c][:, j : j + 1],
                                None,
                                op0=mybir.AluOpType.add,
                            )
                            au = a.bitcast(mybir.dt.uint16)
                            nc.vector.tensor_scalar(
                                au[:],
                                au[:],
                                0x7FFF,
                                None,
                                op0=mybir.AluOpType.bitwise_and,
                            )
                        # chunk c's selector scatters its 16 o-groups to rows
                        # 16c..16c+15 of the 64-row block; 4 chunks accumulate
                        for lo, hi in HS:
                            nc.tensor.matmul(
                                ps[r0 : r0 + 64, lo:hi],
                                sel_sb[:, 64 * c : 64 * (c + 1)],
                                a[:, lo:hi],
                                start=(c == 0),
                                stop=(c == 3),
                            )
                e = epool.tile([P, IW], BF16, tag="E")
                nc.scalar.activation(
                    e[:],
                    ps[:],
                    mybir.ActivationFunctionType.Exp,
                    bias=0.0,
                    scale=-1.0,
                    accum_out=oxacc[:, pr : pr + 1],
                )
                # transpose contributions for the d=1..3 i-blocks: fold the
                # two j-halves and accumulate over all pairs on the PE
                nc.tensor.matmul(
                    psum_s[:, :],
                    sel_sb[:, 256:320],
                    e[:, 128:512],
                    start=(pr == 0),
                    stop=(pr == 63),
                )

            sacc_sb = cpool.tile([64, 384], F32)
            nc.vector.tensor_copy(sacc_sb[:], psum_s[:])
            nc.sync.dma_start(out=ox_out, in_=oxacc[:])
            nc.sync.dma_start(out=s_out, in_=sacc_sb[:])

    nc.compile()
    return nc


_NC = None


def _get_nc():
    global _NC
    if _NC is None:
        _NC = _build_nc()
    return _NC


def _make_in_maps(x, t):
    x = np.ascontiguousarray(np.asarray(x, dtype=np.float32))
    t16 = np.ascontiguousarray(np.asarray(t, dtype=np.float32).astype(ml_dtypes.bfloat16))
    xtg = np.ascontiguousarray(x.T.astype(ml_dtypes.bfloat16))
    # per-chunk selectors: chunk c maps partition p (= ok - 128c) to o-row
    # 16c + p // KD of the 64-row psum block
    sel = np.zeros((P, 320), dtype=ml_dtypes.bfloat16)
    for c in range(4):
        for g in range(16):
            sel[g * KD : (g + 1) * KD, 64 * c + 16 * c + g] = 1.0
    for pp in range(P):
        sel[pp, 256 + (pp % 64)] = 1.0
    in_maps = []
    for c in range(NCORE):
        in_maps.append(
            {
                "xT": np.ascontiguousarray(np.roll(xtg, -c * JS, axis=1)),
                "xj": np.ascontiguousarray(x[c * JS : (c + 1) * JS]),
                "T": t16,
                "sel": sel,
            }
        )
    return in_maps


def _assemble(results):
    out = np.empty((B, IN_F + OUT_F), dtype=np.float32)
    oX = np.zeros((B, OUT_F), dtype=np.float32)
    for c in range(NCORE):
        r = results[c]
        rows = slice(c * JS, (c + 1) * JS)
        out[rows, :IN_F] = r["xout"]
        oxp = r["oxpair"]  # [128, 64]: rows 0:50 -> j=pr, rows 64:114 -> j=pr+64
        oX[rows] += np.concatenate(
            [oxp[0:OUT_F, :].T, oxp[64 : 64 + OUT_F, :].T], axis=0
        )
        # transpose contributions: sacc[(jsub, o), t] sums exp terms over this
        # core's j rows for local i = 128 + t (the d=1..3 blocks)
        s = r["sacc"]
        s50 = s[0:OUT_F, :].T  # [384, 50]
        g0 = (c + 1) * JS
        for blk in range(3):
            gs = (g0 + blk * JS) % B
            oX[gs : gs + JS] += s50[blk * JS : (blk + 1) * JS]
    out[:, IN_F:] = oX
    return out


def kernel(x, T):
    from concourse.bass_utils import run_bass_kernel_spmd

    nc = _get_nc()
    in_maps = _make_in_maps(x, T)
    res = run_bass_kernel_spmd(nc, in_maps, core_ids=list(range(NCORE)))
    return _assemble(res.results)


def _ensure_ntff_hook():
    """The agent image's antenv lacks axon_hooks; synthesize it from the
    ctypes NTFF driver in trn_agent_boot so trace=True works."""
    import sys
    import types

    try:
        from antenv.axon_hooks import get_axon_ntff_profile_hook  # noqa: F401

        return
    except ImportError:
        pass
    from trn_agent_boot.trn_boot import _ntff_profile_via_ctypes

    hook = _ntff_profile_via_ctypes("/opt/axon/libaxon_pjrt.so")
    mod = types.ModuleType("antenv.axon_hooks")
    mod.get_axon_ntff_profile_hook = lambda: hook
    mod.set_axon_ntff_profile_hook = lambda h: None
    sys.modules["antenv.axon_hooks"] = mod


def kernel_profiled(x, T, tmpdir=None):
    """Same as kernel() but with NTFF tracing; returns (out, exec_time_ns)."""
    import concourse.bass_utils as bu

    _ensure_ntff_hook()
    bu.upload_artifacts = lambda d: d  # no S3 in this container

    nc = _get_nc()
    in_maps = _make_in_maps(x, T)
    res = bu.run_bass_kernel_spmd(
        nc, in_maps, core_ids=list(range(NCORE)), trace=True, tmpdir=tmpdir
    )
    return _assemble(res.results), res.exec_time_ns


# revision 30
# speedup vs baseline: 1.1014x; 1.0613x over previous
"""Trainium2 Bass kernel for nn_MinibatchDiscriminator.

reference:
    M = (x @ T).reshape(B, OUT_F, KD)
    norm[i, j, o] = sum_k |M[i,o,k] - M[j,o,k]|
    oX[j, o] = sum_i exp(-norm[i,j,o])
    out = concat(x, oX, axis=1)

Sharding: batch dim of the j-loop across 8 cores. Each core receives a
batch-rotated copy of x^T (so its own 128 j-rows are always M_T columns
0..127 -- one SPMD program serves all cores), computes the full
M_T = (x_rot @ T)^T in [ok, i] layout on the PE.

Symmetry: exp(-norm) is symmetric in (i, j), so each core only computes
i in [0, 640) local (its own diagonal block, neighbours d=1..3, and the
d=4 block which both endpoint cores compute for their own rows). For
d=1..3 the per-(o, i) column sums over the core's j rows are also
accumulated (tile SACC) and redistributed to the i-owning shards during
host-side assembly; the diagonal block contains both (i,j) orders and
the d=4 block is computed by both endpoints, so neither contributes
column sums. Per j:
  |M_T - M_T[:, j]|  split across DVE (tensor_scalar add of the negated
                     column, then a sign-bit bitwise_and on a packed
                     uint32 view -- the ISA has no float abs ALU op) and
                     ACT (Abs activation with per-partition bias)
  k-group reduce     PE matmul with a block-ones selector (partition
                     groups of KD=8 -> OUT_F rows), j-pairs packed two
                     per PSUM tile
  exp + i-sum        single ACT Exp(scale=-1) with accum_out
"""

import ml_dtypes
import numpy as np

import concourse.bacc as bacc
import concourse.bass as bass
import concourse.mybir as mybir
import concourse.tile as tile

B, IN_F, OUT_F, KD = 1024, 1024, 50, 8
OK = OUT_F * KD  # 400
NCORE = 8
JS = B // NCORE  # 128 rows of the batch per core
P = 128
F32 = mybir.dt.float32
BF16 = mybir.dt.bfloat16

# ok-partition chunks: [lo, hi) over the 400 (o,k) pairs, o-major
CHUNKS = [(0, 128), (128, 256), (256, 384), (384, 400)]
IW = 640  # i-range computed per core (5 of 8 blocks, symmetry)
# matmul free-dim slices of the i-range (<=512 each, psum-bank aligned)
HS = [(0, 512), (512, 640)]
# (jsub, chunk) pairs generated on ACT; the rest go to DVE. DVE does
# subtract (bf16, 4x-eligible) + packed sign-mask AND on a uint32 view;
# ACT does Abs(x + bias) in one pass and also owns the exp stage.
ACT_GEN_SETS = [
    {(0, 1), (1, 2), (0, 3)},
    {(0, 1), (1, 2), (1, 0)},
    {(0, 1), (1, 2), (0, 0)},
    {(0, 1), (1, 2)},
]


def _build_nc():
    nc = bacc.Bacc(
        "TRN2",
        target_bir_lowering=False,
        debug=False,
        num_devices=NCORE,
    )
    xT = nc.dram_tensor("xT", [IN_F, IW], BF16, kind="ExternalInput").ap()
    xj = nc.dram_tensor("xj", [JS, IN_F], F32, kind="ExternalInput").ap()
    t_in = nc.dram_tensor("T", [IN_F, OK], BF16, kind="ExternalInput").ap()
    sel_in = nc.dram_tensor("sel", [P, 320], BF16, kind="ExternalInput").ap()
    xout = nc.dram_tensor("xout", [JS, IN_F], F32, kind="ExternalOutput").ap()
    ox_out = nc.dram_tensor("oxpair", [P, 64], F32, kind="ExternalOutput").ap()
    s_out = nc.dram_tensor("sacc", [64, 384], F32, kind="ExternalOutput").ap()

    sub = mybir.AluOpType.subtract

    with tile.TileContext(nc) as tc:
        with (
            tc.tile_pool(name="const", bufs=1) as cpool,
            tc.tile_pool(name="xtp", bufs=1) as xtpool,
            tc.tile_pool(name="agen", bufs=32) as apool,
            tc.tile_pool(name="psn", bufs=3, space=bass.MemorySpace.PSUM) as psn,
            tc.tile_pool(name="spool", bufs=1, space=bass.MemorySpace.PSUM) as spool,
            tc.tile_pool(name="esc", bufs=6) as epool,
        ):
            sel_sb = cpool.tile([P, 320], BF16)
            nc.sync.dma_start(out=sel_sb[:], in_=sel_in)

            t_sb = []
            xt_sb = []
            for fc in range(8):
                tt = cpool.tile([P, OK], BF16, tag=f"t{fc}")
                nc.sync.dma_start(out=tt[:], in_=t_in[fc * 128 : (fc + 1) * 128, :])
                t_sb.append(tt)
                xtt = xtpool.tile([P, IW], BF16, tag=f"xt{fc}")
                nc.sync.dma_start(
                    out=xtt[:, 0:512], in_=xT[fc * 128 : (fc + 1) * 128, 0:512]
                )
                nc.sync.dma_start(
                    out=xtt[:, 512:IW], in_=xT[fc * 128 : (fc + 1) * 128, 512:IW]
                )
                xt_sb.append(xtt)

            # M_T chunks [128, 1024] in bf16 (+ negated copy for the ACT
            # bias path). bf16 is safe: the smallest cross-pair L1 norm is
            # ~50 while exp(-norm) only registers against the exact self
            # term below norm ~16, so +-2 of bf16 noise cannot surface.
            mtb = [cpool.tile([P, IW], BF16, tag=f"mtb{c}", name=f"mtb{c}") for c in range(4)]
            # negated fp32 copy OF THE BF16 VALUES (exact upcast) for the
            # per-partition scalar/bias operands, which must be fp32; using
            # raw-fp32 M here would break the exact-zero self term. The DVE
            # path uses op0=add with this negated column.
            nmt32 = [cpool.tile([P, JS], F32, tag=f"nmt32{c}", name=f"nmt32{c}") for c in range(4)]
            # chunk 3 only has 16 live rows; zero the rest so the garbage
            # partitions reduce to norm=0 -> exp(0)=1 in ignored psum rows
            nc.vector.memset(mtb[3][:], 0.0)
            nc.vector.memset(nmt32[3][:], 0.0)

            with tc.tile_pool(name="psmt", bufs=1, space=bass.MemorySpace.PSUM) as psmt:
                for c, (lo, hi) in enumerate(CHUNKS):
                    w = hi - lo
                    for lo2, hi2 in HS:
                        w2 = hi2 - lo2
                        ps = psmt.tile([P, 512], F32, tag="psmt")
                        for fc in range(8):
                            nc.tensor.matmul(
                                ps[0:w, 0:w2],
                                t_sb[fc][:, lo:hi],
                                xt_sb[fc][:, lo2:hi2],
                                start=(fc == 0),
                                stop=(fc == 7),
                            )
                        nc.vector.tensor_copy(mtb[c][0:w, lo2:hi2], ps[0:w, 0:w2])
                    nc.vector.tensor_scalar(
                        nmt32[c][0:w, :], mtb[c][0:w, 0:JS], -1.0, None,
                        op0=mybir.AluOpType.mult,
                    )

            # x passthrough: no deps; emitted post-setup so startup loads own
            # the DMA queues
            nc.sync.dma_start(out=xout, in_=xj)

            oxacc = cpool.tile([P, 64], F32)
            psum_s = spool.tile([64, 384], F32)

            # main loop: j-pairs (p, p+64) share one [128, IW] psum tile
            for pr in range(64):
                ps = psn.tile([P, IW], F32, tag="psn")
                act_set = ACT_GEN_SETS[pr % 4]
                for jsub in range(2):
                    j = pr + 64 * jsub
                    r0 = 64 * jsub
                    for c in range(4):
                        a = apool.tile([P, IW], BF16, tag="A")
                        if (jsub, c) in act_set:
                            nc.scalar.activation(
                                a[:],
                                mtb[c][:],
                                mybir.ActivationFunctionType.Abs,
                                bias=nmt32[c][:, j : j + 1],
                                scale=1.0,
                            )
                        else:
                            nc.vector.tensor_scalar(
                                a[:],
                                mtb[c][:],
                                nmt32[c][:, j : j + 1],
                                None,
                                op0=mybir.AluOpType.add,
                            )
                            au = a.bitcast(mybir.dt.uint16)
                            nc.vector.tensor_scalar(
                                au[:],
                                au[:],
                                0x7FFF,
                                None,
                                op0=mybir.AluOpType.bitwise_and,
                            )
                        # chunk c's selector scatters its 16 o-groups to rows
                        # 16c..16c+15 of the 64-row block; 4 chunks accumulate
                        for lo, hi in HS:
                            nc.tensor.matmul(
                                ps[r0 : r0 + 64, lo:hi],
                                sel_sb[:, 64 * c : 64 * (c + 1)],
                                a[:, lo:hi],
                                start=(c == 0),
                                stop=(c == 3),
                            )
                e = epool.tile([P, IW], BF16, tag="E")
                nc.scalar.activation(
                    e[:],
                    ps[:],
                    mybir.ActivationFunctionType.Exp,
                    bias=0.0,
                    scale=-1.0,
                    accum_out=oxacc[:, pr : pr + 1],
                )
                # transpose contributions for the d=1..3 i-blocks: fold the
                # two j-halves and accumulate over all pairs on the PE
                nc.tensor.matmul(
                    psum_s[:, :],
                    sel_sb[:, 256:320],
                    e[:, 128:512],
                    start=(pr == 0),
                    stop=(pr == 63),
                )

            sacc_sb = cpool.tile([64, 384], F32)
            nc.vector.tensor_copy(sacc_sb[:], psum_s[:])
            nc.sync.dma_start(out=ox_out, in_=oxacc[:])
            nc.sync.dma_start(out=s_out, in_=sacc_sb[:])

    nc.compile()
    return nc


_NC = None


def _get_nc():
    global _NC
    if _NC is None:
        _NC = _build_nc()
    return _NC


def _make_in_maps(x, t):
    x = np.ascontiguousarray(np.asarray(x, dtype=np.float32))
    t16 = np.ascontiguousarray(np.asarray(t, dtype=np.float32).astype(ml_dtypes.bfloat16))
    xtg = np.ascontiguousarray(x.T.astype(ml_dtypes.bfloat16))
    # per-chunk selectors: chunk c maps partition p (= ok - 128c) to o-row
    # 16c + p // KD of the 64-row psum block
    sel = np.zeros((P, 320), dtype=ml_dtypes.bfloat16)
    for c in range(4):
        for g in range(16):
            sel[g * KD : (g + 1) * KD, 64 * c + 16 * c + g] = 1.0
    for pp in range(P):
        sel[pp, 256 + (pp % 64)] = 1.0
    in_maps = []
    for c in range(NCORE):
        in_maps.append(
            {
                "xT": np.ascontiguousarray(np.roll(xtg, -c * JS, axis=1)[:, :IW]),
                "xj": np.ascontiguousarray(x[c * JS : (c + 1) * JS]),
                "T": t16,
                "sel": sel,
            }
        )
    return in_maps


def _assemble(results):
    out = np.empty((B, IN_F + OUT_F), dtype=np.float32)
    oX = np.zeros((B, OUT_F), dtype=np.float32)
    for c in range(NCORE):
        r = results[c]
        rows = slice(c * JS, (c + 1) * JS)
        out[rows, :IN_F] = r["xout"]
        oxp = r["oxpair"]  # [128, 64]: rows 0:50 -> j=pr, rows 64:114 -> j=pr+64
        oX[rows] += np.concatenate(
            [oxp[0:OUT_F, :].T, oxp[64 : 64 + OUT_F, :].T], axis=0
        )
        # transpose contributions: sacc[(jsub, o), t] sums exp terms over this
        # core's j rows for local i = 128 + t (the d=1..3 blocks)
        s = r["sacc"]
        s50 = s[0:OUT_F, :].T  # [384, 50]
        g0 = (c + 1) * JS
        for blk in range(3):
            gs = (g0 + blk * JS) % B
            oX[gs : gs + JS] += s50[blk * JS : (blk + 1) * JS]
    out[:, IN_F:] = oX
    return out


def kernel(x, T):
    from concourse.bass_utils import run_bass_kernel_spmd

    nc = _get_nc()
    in_maps = _make_in_maps(x, T)
    res = run_bass_kernel_spmd(nc, in_maps, core_ids=list(range(NCORE)))
    return _assemble(res.results)


def _ensure_ntff_hook():
    """The agent image's antenv lacks axon_hooks; synthesize it from the
    ctypes NTFF driver in trn_agent_boot so trace=True works."""
    import sys
    import types

    try:
        from antenv.axon_hooks import get_axon_ntff_profile_hook  # noqa: F401

        return
    except ImportError:
        pass
    from trn_agent_boot.trn_boot import _ntff_profile_via_ctypes

    hook = _ntff_profile_via_ctypes("/opt/axon/libaxon_pjrt.so")
    mod = types.ModuleType("antenv.axon_hooks")
    mod.get_axon_ntff_profile_hook = lambda: hook
    mod.set_axon_ntff_profile_hook = lambda h: None
    sys.modules["antenv.axon_hooks"] = mod


def kernel_profiled(x, T, tmpdir=None):
    """Same as kernel() but with NTFF tracing; returns (out, exec_time_ns)."""
    import concourse.bass_utils as bu

    _ensure_ntff_hook()
    bu.upload_artifacts = lambda d: d  # no S3 in this container

    nc = _get_nc()
    in_maps = _make_in_maps(x, T)
    res = bu.run_bass_kernel_spmd(
        nc, in_maps, core_ids=list(range(NCORE)), trace=True, tmpdir=tmpdir
    )
    return _assemble(res.results), res.exec_time_ns


# revision 31
# speedup vs baseline: 1.1157x; 1.0130x over previous
"""Trainium2 Bass kernel for nn_MinibatchDiscriminator.

reference:
    M = (x @ T).reshape(B, OUT_F, KD)
    norm[i, j, o] = sum_k |M[i,o,k] - M[j,o,k]|
    oX[j, o] = sum_i exp(-norm[i,j,o])
    out = concat(x, oX, axis=1)

Sharding: batch dim of the j-loop across 8 cores. Each core receives a
batch-rotated copy of x^T (so its own 128 j-rows are always M_T columns
0..127 -- one SPMD program serves all cores), computes the full
M_T = (x_rot @ T)^T in [ok, i] layout on the PE.

Symmetry: exp(-norm) is symmetric in (i, j), so each core only computes
i in [0, 640) local (its own diagonal block, neighbours d=1..3, and the
d=4 block which both endpoint cores compute for their own rows). For
d=1..3 the per-(o, i) column sums over the core's j rows are also
accumulated (tile SACC) and redistributed to the i-owning shards during
host-side assembly; the diagonal block contains both (i,j) orders and
the d=4 block is computed by both endpoints, so neither contributes
column sums. Per j:
  |M_T - M_T[:, j]|  split across DVE (tensor_scalar add of the negated
                     column, then a sign-bit bitwise_and on a packed
                     uint32 view -- the ISA has no float abs ALU op) and
                     ACT (Abs activation with per-partition bias)
  k-group reduce     PE matmul with a block-ones selector (partition
                     groups of KD=8 -> OUT_F rows), j-pairs packed two
                     per PSUM tile
  exp + i-sum        single ACT Exp(scale=-1) with accum_out
"""

import ml_dtypes
import numpy as np

import concourse.bacc as bacc
import concourse.bass as bass
import concourse.mybir as mybir
import concourse.tile as tile

B, IN_F, OUT_F, KD = 1024, 1024, 50, 8
OK = OUT_F * KD  # 400
NCORE = 8
JS = B // NCORE  # 128 rows of the batch per core
P = 128
F32 = mybir.dt.float32
BF16 = mybir.dt.bfloat16

# ok-partition chunks: [lo, hi) over the 400 (o,k) pairs, o-major
CHUNKS = [(0, 128), (128, 256), (256, 384), (384, 400)]
IW = 640  # i-range computed per core (5 of 8 blocks, symmetry)
# matmul free-dim slices of the i-range (<=512 each, psum-bank aligned)
HS = [(0, 512), (512, 640)]
# (jsub, chunk) pairs generated on ACT; the rest go to DVE. DVE does
# subtract (bf16, 4x-eligible) + packed sign-mask AND on a uint32 view;
# ACT does Abs(x + bias) in one pass and also owns the exp stage.
ACT_GEN_SETS = [
    {(0, 1), (1, 2), (0, 3)},
    {(0, 1), (1, 2), (1, 0)},
    {(0, 1), (1, 2), (0, 0)},
    {(0, 1), (1, 2)},
]


def _build_nc():
    nc = bacc.Bacc(
        "TRN2",
        target_bir_lowering=False,
        debug=False,
        num_devices=NCORE,
    )
    xT = nc.dram_tensor("xT", [IN_F, IW], BF16, kind="ExternalInput").ap()
    xj = nc.dram_tensor("xj", [JS, IN_F], F32, kind="ExternalInput").ap()
    t_in = nc.dram_tensor("T", [IN_F, OK], BF16, kind="ExternalInput").ap()
    sel_in = nc.dram_tensor("sel", [P, 320], BF16, kind="ExternalInput").ap()
    xout = nc.dram_tensor("xout", [JS, IN_F], F32, kind="ExternalOutput").ap()
    ox_out = nc.dram_tensor("oxpair", [P, 64], F32, kind="ExternalOutput").ap()
    s_out = nc.dram_tensor("sacc", [64, 384], F32, kind="ExternalOutput").ap()

    sub = mybir.AluOpType.subtract

    with tile.TileContext(nc) as tc:
        with (
            tc.tile_pool(name="const", bufs=1) as cpool,
            tc.tile_pool(name="xtp", bufs=1) as xtpool,
            tc.tile_pool(name="agen", bufs=32) as apool,
            tc.tile_pool(name="psn", bufs=3, space=bass.MemorySpace.PSUM) as psn,
            tc.tile_pool(name="esc", bufs=6) as epool,
        ):
            sel_sb = cpool.tile([P, 320], BF16)
            nc.sync.dma_start(out=sel_sb[:], in_=sel_in)

            t_sb = []
            xt_sb = []
            for fc in range(8):
                tt = cpool.tile([P, OK], BF16, tag=f"t{fc}")
                nc.sync.dma_start(out=tt[:], in_=t_in[fc * 128 : (fc + 1) * 128, :])
                t_sb.append(tt)
                xtt = xtpool.tile([P, IW], BF16, tag=f"xt{fc}")
                nc.sync.dma_start(
                    out=xtt[:, 0:512], in_=xT[fc * 128 : (fc + 1) * 128, 0:512]
                )
                nc.sync.dma_start(
                    out=xtt[:, 512:IW], in_=xT[fc * 128 : (fc + 1) * 128, 512:IW]
                )
                xt_sb.append(xtt)

            # M_T chunks [128, 1024] in bf16 (+ negated copy for the ACT
            # bias path). bf16 is safe: the smallest cross-pair L1 norm is
            # ~50 while exp(-norm) only registers against the exact self
            # term below norm ~16, so +-2 of bf16 noise cannot surface.
            mtb = [cpool.tile([P, IW], BF16, tag=f"mtb{c}", name=f"mtb{c}") for c in range(4)]
            # negated fp32 copy OF THE BF16 VALUES (exact upcast) for the
            # per-partition scalar/bias operands, which must be fp32; using
            # raw-fp32 M here would break the exact-zero self term. The DVE
            # path uses op0=add with this negated column.
            nmt32 = [cpool.tile([P, JS], F32, tag=f"nmt32{c}", name=f"nmt32{c}") for c in range(4)]
            # chunk 3 only has 16 live rows; zero the rest so the garbage
            # partitions reduce to norm=0 -> exp(0)=1 in ignored psum rows
            nc.vector.memset(mtb[3][:], 0.0)
            nc.vector.memset(nmt32[3][:], 0.0)

            if True:
                for c, (lo, hi) in enumerate(CHUNKS):
                    w = hi - lo
                    for lo2, hi2 in HS:
                        w2 = hi2 - lo2
                        ps = psn.tile([P, 512], F32, tag="psmt", bufs=2)
                        for fc in range(8):
                            nc.tensor.matmul(
                                ps[0:w, 0:w2],
                                t_sb[fc][:, lo:hi],
                                xt_sb[fc][:, lo2:hi2],
                                start=(fc == 0),
                                stop=(fc == 7),
                            )
                        nc.vector.tensor_copy(mtb[c][0:w, lo2:hi2], ps[0:w, 0:w2])
                    nc.vector.tensor_scalar(
                        nmt32[c][0:w, :], mtb[c][0:w, 0:JS], -1.0, None,
                        op0=mybir.AluOpType.mult,
                    )

            # x passthrough: no deps; emitted post-setup so startup loads own
            # the DMA queues
            nc.sync.dma_start(out=xout, in_=xj)

            oxacc = cpool.tile([P, 64], F32)
            psum_s = psn.tile([64, 384], F32, tag="psmt", bufs=2, name="psum_s")

            # main loop: j-pairs (p, p+64) share one [128, IW] psum tile
            for pr in range(64):
                ps = psn.tile([P, IW], F32, tag="psn")
                act_set = ACT_GEN_SETS[pr % 4]
                for jsub in range(2):
                    j = pr + 64 * jsub
                    r0 = 64 * jsub
                    for c in range(4):
                        a = apool.tile([P, IW], BF16, tag="A")
                        if (jsub, c) in act_set:
                            nc.scalar.activation(
                                a[:],
                                mtb[c][:],
                                mybir.ActivationFunctionType.Abs,
                                bias=nmt32[c][:, j : j + 1],
                                scale=1.0,
                            )
                        else:
                            nc.vector.tensor_scalar(
                                a[:],
                                mtb[c][:],
                                nmt32[c][:, j : j + 1],
                                None,
                                op0=mybir.AluOpType.add,
                            )
                            au = a.bitcast(mybir.dt.uint16)
                            nc.vector.tensor_scalar(
                                au[:],
                                au[:],
                                0x7FFF,
                                None,
                                op0=mybir.AluOpType.bitwise_and,
                            )
                        # chunk c's selector scatters its 16 o-groups to rows
                        # 16c..16c+15 of the 64-row block; 4 chunks accumulate
                        for lo, hi in HS:
                            nc.tensor.matmul(
                                ps[r0 : r0 + 64, lo:hi],
                                sel_sb[:, 64 * c : 64 * (c + 1)],
                                a[:, lo:hi],
                                start=(c == 0),
                                stop=(c == 3),
                            )
                e = epool.tile([P, IW], BF16, tag="E")
                nc.scalar.activation(
                    e[:],
                    ps[:],
                    mybir.ActivationFunctionType.Exp,
                    bias=0.0,
                    scale=-1.0,
                    accum_out=oxacc[:, pr : pr + 1],
                )
                # transpose contributions for the d=1..3 i-blocks: fold the
                # two j-halves and accumulate over all pairs on the PE
                nc.tensor.matmul(
                    psum_s[:, :],
                    sel_sb[:, 256:320],
                    e[:, 128:512],
                    start=(pr == 0),
                    stop=(pr == 63),
                )

            sacc_sb = cpool.tile([64, 384], F32)
            nc.vector.tensor_copy(sacc_sb[:], psum_s[:])
            nc.sync.dma_start(out=ox_out, in_=oxacc[:])
            nc.sync.dma_start(out=s_out, in_=sacc_sb[:])

    nc.compile()
    return nc


_NC = None


def _get_nc():
    global _NC
    if _NC is None:
        _NC = _build_nc()
    return _NC


def _make_in_maps(x, t):
    x = np.ascontiguousarray(np.asarray(x, dtype=np.float32))
    t16 = np.ascontiguousarray(np.asarray(t, dtype=np.float32).astype(ml_dtypes.bfloat16))
    xtg = np.ascontiguousarray(x.T.astype(ml_dtypes.bfloat16))
    # per-chunk selectors: chunk c maps partition p (= ok - 128c) to o-row
    # 16c + p // KD of the 64-row psum block
    sel = np.zeros((P, 320), dtype=ml_dtypes.bfloat16)
    for c in range(4):
        for g in range(16):
            sel[g * KD : (g + 1) * KD, 64 * c + 16 * c + g] = 1.0
    for pp in range(P):
        sel[pp, 256 + (pp % 64)] = 1.0
    in_maps = []
    for c in range(NCORE):
        in_maps.append(
            {
                "xT": np.ascontiguousarray(np.roll(xtg, -c * JS, axis=1)[:, :IW]),
                "xj": np.ascontiguousarray(x[c * JS : (c + 1) * JS]),
                "T": t16,
                "sel": sel,
            }
        )
    return in_maps


def _assemble(results):
    out = np.empty((B, IN_F + OUT_F), dtype=np.float32)
    oX = np.zeros((B, OUT_F), dtype=np.float32)
    for c in range(NCORE):
        r = results[c]
        rows = slice(c * JS, (c + 1) * JS)
        out[rows, :IN_F] = r["xout"]
        oxp = r["oxpair"]  # [128, 64]: rows 0:50 -> j=pr, rows 64:114 -> j=pr+64
        oX[rows] += np.concatenate(
            [oxp[0:OUT_F, :].T, oxp[64 : 64 + OUT_F, :].T], axis=0
        )
        # transpose contributions: sacc[(jsub, o), t] sums exp terms over this
        # core's j rows for local i = 128 + t (the d=1..3 blocks)
        s = r["sacc"]
        s50 = s[0:OUT_F, :].T  # [384, 50]
        g0 = (c + 1) * JS
        for blk in range(3):
            gs = (g0 + blk * JS) % B
            oX[gs : gs + JS] += s50[blk * JS : (blk + 1) * JS]
    out[:, IN_F:] = oX
    return out


def kernel(x, T):
    from concourse.bass_utils import run_bass_kernel_spmd

    nc = _get_nc()
    in_maps = _make_in_maps(x, T)
    res = run_bass_kernel_spmd(nc, in_maps, core_ids=list(range(NCORE)))
    return _assemble(res.results)


def _ensure_ntff_hook():
    """The agent image's antenv lacks axon_hooks; synthesize it from the
    ctypes NTFF driver in trn_agent_boot so trace=True works."""
    import sys
    import types

    try:
        from antenv.axon_hooks import get_axon_ntff_profile_hook  # noqa: F401

        return
    except ImportError:
        pass
    from trn_agent_boot.trn_boot import _ntff_profile_via_ctypes

    hook = _ntff_profile_via_ctypes("/opt/axon/libaxon_pjrt.so")
    mod = types.ModuleType("antenv.axon_hooks")
    mod.get_axon_ntff_profile_hook = lambda: hook
    mod.set_axon_ntff_profile_hook = lambda h: None
    sys.modules["antenv.axon_hooks"] = mod


def kernel_profiled(x, T, tmpdir=None):
    """Same as kernel() but with NTFF tracing; returns (out, exec_time_ns)."""
    import concourse.bass_utils as bu

    _ensure_ntff_hook()
    bu.upload_artifacts = lambda d: d  # no S3 in this container

    nc = _get_nc()
    in_maps = _make_in_maps(x, T)
    res = bu.run_bass_kernel_spmd(
        nc, in_maps, core_ids=list(range(NCORE)), trace=True, tmpdir=tmpdir
    )
    return _assemble(res.results), res.exec_time_ns
